# revision 1
# baseline (speedup 1.0000x reference)
"""Trainium2 Bass kernel for nn_EqPBC (triplet-feature PBC equalizer).

Data-parallel over 8 NeuronCores: each core handles 8192 samples.
Per core, per chunk of 512 samples (batch on free dim, features on partitions):
  1. DMA [128,82] f32 blocks, cast bf16 (DVE), PE-transpose -> E^T [82,512] bf16
  2. One-hot gather matmuls (PE): En/Em/Emn rows (p,h) split (p, h<128|h>=128)
  3. DVE: S1 = sum_p En_p*conj(Emn_p), S2 = sum_p Em_p*conj(Emn_p),
     X_i = Em_i*S1 + En_i*S2  (complex, bf16)
  4. PE reduction over h with W' = W[i,h]*(0.5 on diag) folded into lhsT
  5. f32 finish: out = E[:,L,:] + Eout * 10^(task0/10)/2  (exact f32 E_L term)

Out-of-bounds Emn indices replicate JAX gather semantics: wrap negatives,
then clamp -> both OOB entries land on index 40.
"""
import numpy as np
import ml_dtypes
from contextlib import ExitStack

# ----- static problem constants (hardcoded; kernel.py must be self-contained) -----
M = 41
L = M // 2
NMODES = 2
B = 65536
NCORES = 8
BC = B // NCORES          # 8192 samples per core
NB = 512                  # samples per chunk
NCHUNK = BC // NB         # 16
THRESH = 1.0 * M // 2
_idx = [(m, n) for m in range(-L, L + 1) for n in range(m, L + 1) if abs(m * n) <= THRESH]
M_ARR = np.array([m for m, n in _idx], dtype=np.int32)
N_ARR = np.array([n for m, n in _idx], dtype=np.int32)
DIAG = np.array([m == n for m, n in _idx])
HDIM = len(_idx)          # 177
HA = 128                  # h-split: a block [0,128), b block [128,177)
HB = HDIM - HA            # 49

bf16 = ml_dtypes.bfloat16


def _gather_cols(idx_arr):
    """Column indices into E^T[82,:] (row f = 2*m + p) for gathered rows (p,h)."""
    src = np.empty((2, HDIM), dtype=np.int64)
    for p in range(2):
        src[p] = 2 * (L + idx_arr) + p
    return src  # [p, h] -> source row in [0,82)


BP = 113   # packed b-block rows: p0 tail at 0:49, p1 tail at 64:113
GCOLS = 128 + BP + 128  # 369


def _build_consts():
    mn = L + M_ARR + N_ARR
    mn = np.clip(np.where(mn < 0, mn + M, mn), 0, M - 1) - L  # jax wrap+clamp
    srcs = {"n": _gather_cols(N_ARR), "m": _gather_cols(M_ARR), "mn": _gather_cols(mn)}
    gmats = {}
    for k, src in srcs.items():
        G = np.zeros((82, GCOLS), dtype=np.float32)
        for p in range(2):
            for h in range(HA):                      # a-blocks
                G[src[p, h], (0 if p == 0 else 128 + BP) + h] = 1.0
            for r in range(HB):                      # packed b-block
                G[src[p, HA + r], 128 + (0 if p == 0 else 64) + r] = 1.0
        gmats[k] = G.astype(bf16)
    return gmats


def _build_wred(Wr, Wi):
    """[177, 8] bf16: cols (i*4+0,1) = (W'r,W'i) for rhs=X_ir;
    cols (i*4+2,3) = (-W'i, W'r) for rhs=X_ii.  W' = W[i]*(0.5 on diag)."""
    scale = np.where(DIAG, 0.5, 1.0).astype(np.float32)
    out = np.zeros((HDIM, 8), dtype=np.float32)
    for i in range(2):
        wr = Wr[i] * scale
        wi = Wi[i] * scale
        out[:, i * 4 + 0] = wr
        out[:, i * 4 + 1] = wi
        out[:, i * 4 + 2] = -wi
        out[:, i * 4 + 3] = wr
    return out.astype(bf16)


def _build_ffold():
    """[113,113] bf16: out[j] = in[j] + in[64+j] for j<49, replicated at 64+j;
    pad cols 49:64 zero."""
    F = np.zeros((BP, BP), dtype=np.float32)
    for h in range(HB):
        for r in (h, 64 + h):
            F[r, h] = 1.0
            F[r, 64 + h] = 1.0
    return F.astype(bf16)


def _build_wredb(Wr, Wi):
    """[113, 8] bf16 for the packed b-block: mode0 tail at rows 0:49,
    mode1 tail at rows 64:113; col layout as in _build_wred."""
    scale = np.where(DIAG, 0.5, 1.0).astype(np.float32)
    out = np.zeros((BP, 8), dtype=np.float32)
    for i in range(2):
        wr = (Wr[i] * scale)[HA:]
        wi = (Wi[i] * scale)[HA:]
        r0 = 0 if i == 0 else 64
        out[r0:r0 + HB, i * 4 + 0] = wr
        out[r0:r0 + HB, i * 4 + 1] = wi
        out[r0:r0 + HB, i * 4 + 2] = -wi
        out[r0:r0 + HB, i * 4 + 3] = wr
    return out.astype(bf16)


def _build_kernel():
    import concourse.bass as bass
    import concourse.bacc as bacc
    import concourse.tile as tile
    import concourse.mybir as mybir

    dt = mybir.dt
    nc = bacc.Bacc("TRN2", target_bir_lowering=False, debug=False, num_devices=NCORES)
    xr = nc.declare_dram_parameter("xr", [BC, 82], dt.float32, isOutput=False)
    xi = nc.declare_dram_parameter("xi", [BC, 82], dt.float32, isOutput=False)
    ti = nc.declare_dram_parameter("ti", [BC, 4], dt.float32, isOutput=False)
    gn_d = nc.declare_dram_parameter("gn", [82, GCOLS], dt.bfloat16, isOutput=False)
    gm_d = nc.declare_dram_parameter("gm", [82, GCOLS], dt.bfloat16, isOutput=False)
    gmn_d = nc.declare_dram_parameter("gmn", [82, GCOLS], dt.bfloat16, isOutput=False)
    wred_d = nc.declare_dram_parameter("wred", [HDIM, 8], dt.bfloat16, isOutput=False)
    wredb_d = nc.declare_dram_parameter("wredb", [BP, 8], dt.bfloat16, isOutput=False)
    ffold_d = nc.declare_dram_parameter("ffold", [BP, BP], dt.bfloat16, isOutput=False)
    id128_d = nc.declare_dram_parameter("id128", [128, 128], dt.float32, isOutput=False)
    id4_d = nc.declare_dram_parameter("id4", [2, 2], dt.float32, isOutput=False)
    out_d = nc.declare_dram_parameter("out", [BC, 4], dt.float32, isOutput=True)

    LN10_10 = float(np.log(10.0) / 10.0)
    LNHALF = float(np.log(0.5))

    with tile.TileContext(nc) as tc, ExitStack() as ctx:
        cpool = ctx.enter_context(tc.tile_pool(name="consts", bufs=1))
        nat_pool = ctx.enter_context(tc.tile_pool(name="nat", bufs=6))
        et_pool = ctx.enter_context(tc.tile_pool(name="et", bufs=3))
        g_pool = ctx.enter_context(tc.tile_pool(name="gath", bufs=2))
        s_pool = ctx.enter_context(tc.tile_pool(name="smid", bufs=3))
        tmp_pool = ctx.enter_context(tc.tile_pool(name="tmps", bufs=2))
        x_pool = ctx.enter_context(tc.tile_pool(name="xmid", bufs=3))
        e_pool = ctx.enter_context(tc.tile_pool(name="eall", bufs=2))
        o_pool = ctx.enter_context(tc.tile_pool(name="outs", bufs=2))
        pt_psum = ctx.enter_context(tc.tile_pool(name="ptp", bufs=1, space="PSUM"))
        pg_psum = ctx.enter_context(tc.tile_pool(name="pgp", bufs=4, space="PSUM"))
        pe_psum = ctx.enter_context(tc.tile_pool(name="pep", bufs=2, space="PSUM"))
        po_psum = ctx.enter_context(tc.tile_pool(name="pop", bufs=1, space="PSUM"))

        # load constants once
        gmats_sb = {}
        for name, d in (("n", gn_d), ("m", gm_d), ("mn", gmn_d)):
            t = cpool.tile([82, GCOLS], dt.bfloat16, tag=f"g{name}")
            nc.gpsimd.dma_start(out=t[:], in_=d[:])
            gmats_sb[name] = t
        wredA = cpool.tile([HA, 8], dt.bfloat16, tag="wredA")
        nc.gpsimd.dma_start(out=wredA[:], in_=wred_d[0:HA, :])
        wredB = cpool.tile([BP, 8], dt.bfloat16, tag="wredB")
        nc.gpsimd.dma_start(out=wredB[:], in_=wredb_d[:])
        id128 = cpool.tile([128, 128], dt.float32, tag="id128")
        nc.gpsimd.dma_start(out=id128[:], in_=id128_d[:])
        id4 = cpool.tile([2, 2], dt.float32, tag="id4")
        nc.gpsimd.dma_start(out=id4[:], in_=id4_d[:])
        ffold = cpool.tile([BP, BP], dt.bfloat16, tag="ffold")
        nc.gpsimd.dma_start(out=ffold[:], in_=ffold_d[:])
        bias_t = cpool.tile([128, 1], dt.float32, tag="biasln")
        nc.vector.memset(bias_t[:], LNHALF)

        # J-slices of gather matrices: [a0(p0 h<128), bpack(113), a1(p1 h<128)]
        jslices = [(0, HA), (HA, BP), (HA + BP, HA)]

        for c in range(NCHUNK):
            b0 = c * NB
            nat = {}
            etT = {}
            for comp, src in (("r", xr), ("i", xi)):
                et = et_pool.tile([82, NB], dt.bfloat16, tag=f"et{comp}")
                etT[comp] = et
                for blk in range(4):
                    t = nat_pool.tile([128, 82], dt.float32, tag=f"nat{comp}")
                    nc.gpsimd.dma_start(out=t[:], in_=src[b0 + blk * 128: b0 + (blk + 1) * 128, :])
                    if blk == 3:
                        nat[comp] = t  # keep last block for E_L columns (see below)
                    nat[(comp, blk)] = t
                    pt = pt_psum.tile([82, 128], dt.float32, tag="tpsum")
                    nc.tensor.transpose(pt[:], t[:], id128[:])
                    nc.scalar.copy(et[:, blk * 128:(blk + 1) * 128], pt[:])

            # gathers: gtile[kind][comp][j] with j in 0..3 = (p0a,p0b,p1a,p1b)
            gt = {}
            for kind in ("n", "m", "mn"):
                for comp in ("r", "i"):
                    for j, (j0, jl) in enumerate(jslices):
                        ps = pg_psum.tile([128, NB], dt.float32, tag="gpsum")
                        nc.tensor.matmul(ps[:jl, :], gmats_sb[kind][:, j0:j0 + jl],
                                         etT[comp][:], start=True, stop=True)
                        sb = g_pool.tile([128, NB], dt.bfloat16, tag=f"g{kind}{comp}{j}")
                        nc.scalar.copy(sb[:jl, :], ps[:jl, :])
                        gt[(kind, comp, j)] = sb

            def TT(op, out, a, b_, rows, eng=None):
                getattr(eng or nc.vector, op)(out[:rows, :], a[:rows, :], b_[:rows, :])

            # S-stage over 3 gathered tiles: j=0 a0(p0,128), j=1 bpack(113: p0
            # tail at 0:49, p1 tail at 64:113), j=2 a1(p1,128).
            S = {}
            for (sname, kind) in (("S1", "n"), ("S2", "m")):
                for comp in ("r", "i"):
                    prods = {}
                    for j, rows in ((0, HA), (1, BP), (2, HA)):
                        pa = tmp_pool.tile([128, NB], dt.bfloat16, tag=f"pa{j}")
                        pb_ = tmp_pool.tile([128, NB], dt.bfloat16, tag=f"pb{j}")
                        if comp == "r":
                            TT("tensor_mul", pa, gt[(kind, "r", j)], gt[("mn", "r", j)], rows)
                            TT("tensor_mul", pb_, gt[(kind, "i", j)], gt[("mn", "i", j)], rows)
                        else:
                            TT("tensor_mul", pa, gt[(kind, "i", j)], gt[("mn", "r", j)], rows)
                            TT("tensor_mul", pb_, gt[(kind, "r", j)], gt[("mn", "i", j)], rows)
                        prods[j] = (pa, pb_)
                    qs = {}
                    for j, rows in ((0, HA), (1, BP), (2, HA)):
                        q = tmp_pool.tile([128, NB], dt.bfloat16, tag=f"q{j}")
                        TT("tensor_add" if comp == "r" else "tensor_sub", q, prods[j][0], prods[j][1], rows)
                        qs[j] = q
                    sa = s_pool.tile([128, NB], dt.bfloat16, tag=f"{sname}{comp}a")
                    TT("tensor_add", sa, qs[0], qs[2], HA)
                    # packed b fold: S_b[r] = q1[r] + q1[64+r]; write it at BOTH
                    # row offsets so it aligns with either mode's packed operand
                    psf = pg_psum.tile([128, NB], dt.float32, tag="gpsum")
                    nc.tensor.matmul(psf[:BP, :], ffold[:], qs[1][:BP, :], start=True, stop=True)
                    sbp = s_pool.tile([128, NB], dt.bfloat16, tag=f"{sname}{comp}b")
                    nc.scalar.copy(sbp[:BP, :], psf[:BP, :])
                    S[(sname, comp, 0)] = sa
                    S[(sname, comp, 1)] = sbp

            # X-stage: a-blocks per mode i (gt j = 0 or 2); b-block packed for
            # both modes at once (gt j = 1, lhsT weights select the mode rows).
            X = {}
            for comp in ("r", "i"):
                s1a, s1b = "S1", "S2"
                ops = []  # (out_key, gkind_tiles_j, rows)
                for sel in (0, 2, 1):  # a0 (i=0), a1 (i=1), bpack (both)
                    rows = BP if sel == 1 else HA
                    hb = 1 if sel == 1 else 0
                    t1 = tmp_pool.tile([128, NB], dt.bfloat16, tag=f"xt1{sel}")
                    t2 = tmp_pool.tile([128, NB], dt.bfloat16, tag=f"xt2{sel}")
                    t3 = tmp_pool.tile([128, NB], dt.bfloat16, tag=f"xt3{sel}")
                    t4 = tmp_pool.tile([128, NB], dt.bfloat16, tag=f"xt4{sel}")
                    if comp == "r":
                        TT("tensor_mul", t1, gt[("m", "r", sel)], S[("S1", "r", hb)], rows)
                        TT("tensor_mul", t2, gt[("m", "i", sel)], S[("S1", "i", hb)], rows)
                        TT("tensor_mul", t3, gt[("n", "r", sel)], S[("S2", "r", hb)], rows)
                        TT("tensor_mul", t4, gt[("n", "i", sel)], S[("S2", "i", hb)], rows)
                    else:
                        TT("tensor_mul", t1, gt[("m", "r", sel)], S[("S1", "i", hb)], rows)
                        TT("tensor_mul", t2, gt[("m", "i", sel)], S[("S1", "r", hb)], rows)
                        TT("tensor_mul", t3, gt[("n", "r", sel)], S[("S2", "i", hb)], rows)
                        TT("tensor_mul", t4, gt[("n", "i", sel)], S[("S2", "r", hb)], rows)
                    ops.append((sel, rows, t1, t2, t3, t4))
                uv = {}
                for sel, rows, t1, t2, t3, t4 in ops:
                    u = tmp_pool.tile([128, NB], dt.bfloat16, tag=f"xu{sel}")
                    v = tmp_pool.tile([128, NB], dt.bfloat16, tag=f"xv{sel}")
                    if comp == "r":
                        TT("tensor_sub", u, t1, t2, rows)
                        TT("tensor_sub", v, t3, t4, rows)
                    else:
                        TT("tensor_add", u, t1, t2, rows)
                        TT("tensor_add", v, t3, t4, rows)
                    uv[sel] = (u, v, rows)
                for sel in (0, 2, 1):
                    u, v, rows = uv[sel]
                    xt = x_pool.tile([128, NB], dt.bfloat16, tag=f"x{comp}{sel}")
                    TT("tensor_add", xt, u, v, rows)
                    X[(comp, sel)] = xt

            # reduction: Eout_i = sum_h W'_i[h] * X_i[h]; a-block per mode
            # (X[(comp, 0|2)]) + packed-b (X[(comp,1)], wredB rows select mode)
            eall0 = e_pool.tile([2, NB], dt.float32, tag="eall0")
            eall1 = e_pool.tile([2, NB], dt.float32, tag="eall1")
            eall = [eall0, eall1]
            for i in range(2):
                sel = 0 if i == 0 else 2
                pe = pe_psum.tile([2, NB], dt.float32, tag="epsum")
                nc.tensor.matmul(pe[:], wredA[:, i * 4:i * 4 + 2], X[("r", sel)][:HA, :],
                                 start=True, stop=False)
                nc.tensor.matmul(pe[:], wredA[:, i * 4 + 2:i * 4 + 4], X[("i", sel)][:HA, :],
                                 start=False, stop=False)
                nc.tensor.matmul(pe[:], wredB[:, i * 4:i * 4 + 2], X[("r", 1)][:BP, :],
                                 start=False, stop=False)
                nc.tensor.matmul(pe[:], wredB[:, i * 4 + 2:i * 4 + 4], X[("i", 1)][:BP, :],
                                 start=False, stop=True)
                nc.scalar.copy(eall[i][:], pe[:])

            # final combine per 128-block
            for blk in range(4):
                po = po_psum.tile([128, 4], dt.float32, tag="opsum")
                nc.tensor.transpose(po[:, 0:2], eall[0][:, blk * 128:(blk + 1) * 128], id4[:])
                nc.tensor.transpose(po[:, 2:4], eall[1][:, blk * 128:(blk + 1) * 128], id4[:])
                tit = o_pool.tile([128, 4], dt.float32, tag="tit")
                nc.gpsimd.dma_start(out=tit[:], in_=ti[b0 + blk * 128: b0 + (blk + 1) * 128, :])
                pcol = o_pool.tile([128, 1], dt.float32, tag="pcol")
                import concourse.mybir as _mb
                nc.scalar.activation(pcol[:], tit[:, 0:1], _mb.ActivationFunctionType.Exp,
                                     bias=bias_t[:], scale=LN10_10)
                ot = o_pool.tile([128, 4], dt.float32, tag="ot")
                nc.vector.tensor_scalar_mul(ot[:], po[:], pcol[:])
                # add exact E_L columns: out cols (0,2) += xr_nat[:, 40:42]; (1,3) += xi_nat
                nc.vector.tensor_add(ot[:, 0:4:2], ot[:, 0:4:2], nat[("r", blk)][:, 2 * L:2 * L + 2])
                nc.vector.tensor_add(ot[:, 1:4:2], ot[:, 1:4:2], nat[("i", blk)][:, 2 * L:2 * L + 2])
                nc.sync.dma_start(out=out_d[b0 + blk * 128: b0 + (blk + 1) * 128, :], in_=ot[:])

    nc.compile()
    return nc


_CACHE = {}


def kernel(xr, xi, task_info, Wr, Wi):
    from concourse.bass_utils import run_bass_kernel_spmd

    xr = np.ascontiguousarray(np.asarray(xr, dtype=np.float32)).reshape(B, 82)
    xi = np.ascontiguousarray(np.asarray(xi, dtype=np.float32)).reshape(B, 82)
    task_info = np.ascontiguousarray(np.asarray(task_info, dtype=np.float32))
    gm = _build_consts()
    Wr32 = np.asarray(Wr, dtype=np.float32); Wi32 = np.asarray(Wi, dtype=np.float32)
    wred = _build_wred(Wr32, Wi32)
    wredb = _build_wredb(Wr32, Wi32)
    id128 = np.eye(128, dtype=np.float32)
    id4 = np.eye(2, dtype=np.float32)

    if "nc" not in _CACHE:
        _CACHE["nc"] = _build_kernel()
    nc = _CACHE["nc"]

    in_maps = []
    for core in range(NCORES):
        s = slice(core * BC, (core + 1) * BC)
        in_maps.append({
            "xr": xr[s], "xi": xi[s], "ti": task_info[s],
            "gn": gm["n"], "gm": gm["m"], "gmn": gm["mn"],
            "wred": wred, "wredb": wredb, "ffold": _build_ffold(), "id128": id128, "id4": id4,
        })
    res = run_bass_kernel_spmd(nc, in_maps, list(range(NCORES)))
    outs = [res.results[i]["out"] for i in range(NCORES)]
    full = np.concatenate(outs, axis=0)  # [B, 4]
    return full.reshape(B, NMODES, 2).astype(np.float32)



# revision 8
# speedup vs baseline: 1.3836x; 1.3836x over previous
"""Trainium2 Bass kernel for nn_EqPBC (triplet-feature PBC equalizer).

Data-parallel over 8 NeuronCores: each core handles 8192 samples.
Per core, per chunk of 512 samples (features on partitions, batch on free):
  1. One HWDGE DMA per input per chunk: [512,82] f32 viewed as [128,328]
     (partition p holds samples 4p..4p+3), PE-transpose 4 slices into one
     PSUM tile -> E^T [82,512] bf16.
  2. Gather matmuls (PE) with one-hot G [82,384]: concatenated layout
     [a0(p0,h<128) | pack(113 tail rows, zero-padded to 128) | a1(p1,h<128)]
     -> 6 tiles [128,1536] bf16 (En,Em,Emn x r,i).
  3. S-stage (DVE): per S-comp only 2 muls @1536 (pa,pb); the p-fold is
     3 adds @512 on the a-blocks and TWO accumulating ffold matmuls
     (+F for pa, +/-F for pb) on the packed block (PE absorbs the add).
  4. X-stage: 8 product muls (split 1024+512 so S needs no dup copy),
     all +/- recombination folded into the PE reduction via signed
     weight-class lhsT columns; 24 accumulating matmuls -> [4,512] PSUM.
  5. f32 finish: transpose [4,512] -> [128,16], out = E_L + Eout * P,
     one linear HWDGE DMA per chunk.

Out-of-bounds Emn indices replicate JAX gather semantics (wrap, clamp).
"""
import numpy as np
import ml_dtypes
from contextlib import ExitStack

# ----- static problem constants (hardcoded; kernel.py must be self-contained) -----
M = 41
L = M // 2
NMODES = 2
B = 65536
NCORES = 8
BC = B // NCORES          # 8192 samples per core
NB = 512                  # samples per chunk
NCHUNK = BC // NB         # 16
THRESH = 1.0 * M // 2
_idx = [(m, n) for m in range(-L, L + 1) for n in range(m, L + 1) if abs(m * n) <= THRESH]
M_ARR = np.array([m for m, n in _idx], dtype=np.int32)
N_ARR = np.array([n for m, n in _idx], dtype=np.int32)
DIAG = np.array([m == n for m, n in _idx])
HDIM = len(_idx)          # 177
HA = 128                  # h-split: a block [0,128), tail [128,177)
HB = HDIM - HA            # 49

bf16 = ml_dtypes.bfloat16

GC = 384                  # gather cols: [a0 128 | pack 128 (113 used) | a1 128]


def _gather_cols(idx_arr):
    """Row index into E^T[82,:] (row f = 2*(L+idx) + p) for gathered (p,h)."""
    src = np.empty((2, HDIM), dtype=np.int64)
    for p in range(2):
        src[p] = 2 * (L + idx_arr) + p
    return src


def _build_consts():
    mn = L + M_ARR + N_ARR
    mn = np.clip(np.where(mn < 0, mn + M, mn), 0, M - 1) - L  # jax wrap+clamp
    srcs = {"n": _gather_cols(N_ARR), "m": _gather_cols(M_ARR), "mn": _gather_cols(mn)}
    gmats = {}
    for k, src in srcs.items():
        G = np.zeros((82, GC), dtype=np.float32)
        for p in range(2):
            for h in range(HA):                      # a-blocks
                G[src[p, h], (0 if p == 0 else 256) + h] = 1.0
            for r in range(HB):                      # packed tail block
                G[src[p, HA + r], 128 + (0 if p == 0 else 64) + r] = 1.0
        gmats[k] = G.astype(bf16)
    return gmats


def _build_ffold(sign):
    """[128,128] bf16: out[c] = sign*(in[c'] contributions) with
    out[r] = in[r] + in[64+r] for r in 0:49 and replicated at 64+r."""
    F = np.zeros((128, 128), dtype=np.float32)
    for h in range(HB):
        for c in (h, 64 + h):
            F[h, c] = sign
            F[64 + h, c] = sign
    return F.astype(bf16)


def _build_wall(Wr, Wi):
    """[128, 36] bf16: 9 col-groups of 4 = (cls in p,m,i) x (blk in A0,P,A1).
    Group order: A0p A0m A0i Pp Pm Pi A1p A1m A1i.
    lhsT [128,4] cols = (mode0 r, mode0 i, mode1 r, mode1 i) out rows.
    cls p: (wr, wi); m: (-wr, -wi); i: (-wi, wr). W' = W*(0.5 on diag)."""
    scale = np.where(DIAG, 0.5, 1.0).astype(np.float32)
    wr = [Wr[i] * scale for i in range(2)]
    wi = [Wi[i] * scale for i in range(2)]

    def cls_cols(i, cls, hsel):
        if cls == "p":
            return wr[i][hsel], wi[i][hsel]
        if cls == "m":
            return -wr[i][hsel], -wi[i][hsel]
        return -wi[i][hsel], wr[i][hsel]

    out = np.zeros((128, 36), dtype=np.float32)
    g = 0
    for blk in ("A0", "P", "A1"):
        for cls in ("p", "m", "i"):
            c0 = g * 4
            if blk == "A0":
                a, b = cls_cols(0, cls, slice(0, HA))
                out[:, c0 + 0] = a
                out[:, c0 + 1] = b
            elif blk == "A1":
                a, b = cls_cols(1, cls, slice(0, HA))
                out[:, c0 + 2] = a
                out[:, c0 + 3] = b
            else:
                a, b = cls_cols(0, cls, slice(HA, HDIM))
                out[0:HB, c0 + 0] = a
                out[0:HB, c0 + 1] = b
                a, b = cls_cols(1, cls, slice(HA, HDIM))
                out[64:64 + HB, c0 + 2] = a
                out[64:64 + HB, c0 + 3] = b
            g += 1
    return out.astype(bf16)


# col-group index into wall for (cls, blk)
_WG = {(c, b): (bi * 3 + ci) * 4
       for bi, b in enumerate(("A0", "P", "A1"))
       for ci, c in enumerate(("p", "m", "i"))}


def _build_kernel():
    import concourse.bass as bass
    import concourse.bacc as bacc
    import concourse.tile as tile
    import concourse.mybir as mybir

    dt = mybir.dt
    nc = bacc.Bacc("TRN2", target_bir_lowering=False, debug=False, num_devices=NCORES)
    xr = nc.declare_dram_parameter("xr", [BC // 4, 328], dt.float32, isOutput=False)
    xi = nc.declare_dram_parameter("xi", [BC // 4, 328], dt.float32, isOutput=False)
    ti = nc.declare_dram_parameter("ti", [BC // 4, 16], dt.float32, isOutput=False)
    gn_d = nc.declare_dram_parameter("gn", [82, GC], dt.bfloat16, isOutput=False)
    gm_d = nc.declare_dram_parameter("gm", [82, GC], dt.bfloat16, isOutput=False)
    gmn_d = nc.declare_dram_parameter("gmn", [82, GC], dt.bfloat16, isOutput=False)
    ffp_d = nc.declare_dram_parameter("ffp", [128, 128], dt.bfloat16, isOutput=False)
    ffm_d = nc.declare_dram_parameter("ffm", [128, 128], dt.bfloat16, isOutput=False)
    wall_d = nc.declare_dram_parameter("wall", [128, 36], dt.bfloat16, isOutput=False)
    id128_d = nc.declare_dram_parameter("id128", [128, 128], dt.float32, isOutput=False)
    id4_d = nc.declare_dram_parameter("id4", [4, 4], dt.float32, isOutput=False)
    out_d = nc.declare_dram_parameter("out", [BC // 4, 16], dt.float32, isOutput=True)

    LN10_10 = float(np.log(10.0) / 10.0)
    LNHALF = float(np.log(0.5))

    with tile.TileContext(nc) as tc, ExitStack() as ctx:
        cpool = ctx.enter_context(tc.tile_pool(name="consts", bufs=1))
        natp = ctx.enter_context(tc.tile_pool(name="nat", bufs=2))
        etp = ctx.enter_context(tc.tile_pool(name="et", bufs=2))
        gp = ctx.enter_context(tc.tile_pool(name="gath", bufs=2))
        sp = ctx.enter_context(tc.tile_pool(name="smid", bufs=2))
        tmpp = ctx.enter_context(tc.tile_pool(name="tmps", bufs=3))
        pp = ctx.enter_context(tc.tile_pool(name="prod", bufs=2))
        ep = ctx.enter_context(tc.tile_pool(name="eall", bufs=2))
        op = ctx.enter_context(tc.tile_pool(name="outs", bufs=2))
        ps_gb = ctx.enter_context(tc.tile_pool(name="pgb", bufs=2, space="PSUM"))
        ps_ga = ctx.enter_context(tc.tile_pool(name="pga", bufs=2, space="PSUM"))
        ps_misc = ctx.enter_context(tc.tile_pool(name="pmisc", bufs=1, space="PSUM"))
        ps_red = ctx.enter_context(tc.tile_pool(name="pred", bufs=1, space="PSUM"))

        # ---- constants (loaded once) ----
        G = {}
        for name, d in (("n", gn_d), ("m", gm_d), ("mn", gmn_d)):
            t = cpool.tile([82, GC], dt.bfloat16, tag=f"g{name}")
            nc.sync.dma_start(out=t[:], in_=d[:])
            G[name] = t
        ffp = cpool.tile([128, 128], dt.bfloat16, tag="ffp")
        nc.sync.dma_start(out=ffp[:], in_=ffp_d[:])
        ffm = cpool.tile([128, 128], dt.bfloat16, tag="ffm")
        nc.sync.dma_start(out=ffm[:], in_=ffm_d[:])
        wall = cpool.tile([128, 36], dt.bfloat16, tag="wall")
        nc.sync.dma_start(out=wall[:], in_=wall_d[:])
        id128 = cpool.tile([128, 128], dt.float32, tag="id128")
        nc.sync.dma_start(out=id128[:], in_=id128_d[:])
        id4 = cpool.tile([4, 4], dt.float32, tag="id4")
        nc.sync.dma_start(out=id4[:], in_=id4_d[:])
        bias_t = cpool.tile([128, 1], dt.float32, tag="biasln")
        nc.vector.memset(bias_t[:], LNHALF)

        import concourse.mybir as _mb

        for c in range(NCHUNK):
            r0 = c * 128  # row offset into the [BC//4, *] dram views

            # ---- stage A: load + transpose + cast ----
            nat = {}
            et = {}
            for comp, src in (("r", xr), ("i", xi)):
                t = natp.tile([128, 328], dt.float32, tag=f"nat{comp}")
                nc.sync.dma_start(out=t[:], in_=src[r0:r0 + 128, :])
                nat[comp] = t
                pt = ps_misc.tile([128, 512], dt.float32, tag="misc")
                for s in range(4):
                    nc.tensor.transpose(pt[0:82, s * 128:(s + 1) * 128],
                                        t[:, s * 82:(s + 1) * 82], id128[:])
                e = etp.tile([82, 512], dt.bfloat16, tag=f"et{comp}")
                nc.scalar.copy(e[:], pt[0:82, :])
                et[comp] = e
            tit = op.tile([128, 16], dt.float32, tag="tit")
            nc.sync.dma_start(out=tit[:], in_=ti[r0:r0 + 128, :])

            # ---- stage B: gathers -> [128,1536] bf16 (layout a0|pack|a1) ----
            gt = {}
            for kind in ("n", "m", "mn"):
                for comp in ("r", "i"):
                    pgb = ps_gb.tile([128, 1024], dt.float32, tag="pgb")
                    nc.tensor.matmul(pgb[:, 0:512], G[kind][:, 0:128], et[comp][:],
                                     start=True, stop=True)
                    nc.tensor.matmul(pgb[:, 512:1024], G[kind][:, 128:256], et[comp][:],
                                     start=True, stop=True)
                    pga = ps_ga.tile([128, 512], dt.float32, tag="pga")
                    nc.tensor.matmul(pga[:], G[kind][:, 256:384], et[comp][:],
                                     start=True, stop=True)
                    g = gp.tile([128, 1536], dt.bfloat16, tag=f"g{kind}{comp}")
                    nc.scalar.copy(g[:, 0:1024], pgb[:])
                    nc.scalar.copy(g[:, 1024:1536], pga[:])
                    gt[(kind, comp)] = g

            # ---- stage C: S tensors [128,1024] = [S_a | S_pack] ----
            S = {}
            for sname, kind in (("S1", "n"), ("S2", "m")):
                for comp in ("r", "i"):
                    pa = tmpp.tile([128, 1536], dt.bfloat16, tag="pa")
                    pb = tmpp.tile([128, 1536], dt.bfloat16, tag="pb")
                    if comp == "r":
                        # S_r = kr*mnr + ki*mni
                        nc.vector.tensor_mul(pa[:], gt[(kind, "r")][:], gt[("mn", "r")][:])
                        nc.vector.tensor_mul(pb[:], gt[(kind, "i")][:], gt[("mn", "i")][:])
                        sign = 1
                    else:
                        # S_i = ki*mnr - kr*mni
                        nc.vector.tensor_mul(pa[:], gt[(kind, "i")][:], gt[("mn", "r")][:])
                        nc.vector.tensor_mul(pb[:], gt[(kind, "r")][:], gt[("mn", "i")][:])
                        sign = -1
                    st = sp.tile([128, 1024], dt.bfloat16, tag=f"{sname}{comp}")
                    # a-block p-fold: S_a = (pa0 + pa2) +/- (pb0 + pb2)
                    u = tmpp.tile([128, 512], dt.bfloat16, tag="fu")
                    v = tmpp.tile([128, 512], dt.bfloat16, tag="fv")
                    nc.vector.tensor_add(u[:], pa[:, 0:512], pa[:, 1024:1536])
                    nc.vector.tensor_add(v[:], pb[:, 0:512], pb[:, 1024:1536])
                    if sign > 0:
                        nc.vector.tensor_add(st[:, 0:512], u[:], v[:])
                    else:
                        nc.vector.tensor_sub(st[:, 0:512], u[:], v[:])
                    # packed-block fold on PE: ffold*pa1 +/- ffold*pb1
                    psf = ps_misc.tile([128, 512], dt.float32, tag="misc")
                    nc.tensor.matmul(psf[:], ffp[:], pa[:, 512:1024], start=True, stop=False)
                    nc.tensor.matmul(psf[:], ffp[:] if sign > 0 else ffm[:],
                                     pb[:, 512:1024], start=False, stop=True)
                    nc.scalar.copy(st[:, 512:1024], psf[:])
                    S[(sname, comp)] = st

            # ---- stage D: X products + PE reduction with signed weights ----
            red = ps_red.tile([4, 512], dt.float32, tag="red")
            prods = [
                (("m", "r"), ("S1", "r"), "p"),
                (("m", "i"), ("S1", "i"), "m"),
                (("n", "r"), ("S2", "r"), "p"),
                (("n", "i"), ("S2", "i"), "m"),
                (("m", "r"), ("S1", "i"), "i"),
                (("m", "i"), ("S1", "r"), "i"),
                (("n", "r"), ("S2", "i"), "i"),
                (("n", "i"), ("S2", "r"), "i"),
            ]
            for k, (gk, sk, cls) in enumerate(prods):
                pk = pp.tile([128, 1536], dt.bfloat16, tag=f"P{k}")
                eng = nc.gpsimd if k in (1, 3, 5, 7) else nc.vector
                eng.tensor_mul(pk[:, 0:1024], gt[gk][:, 0:1024], S[sk][:, 0:1024])
                eng.tensor_mul(pk[:, 1024:1536], gt[gk][:, 1024:1536], S[sk][:, 0:512])
                for bi, blk in enumerate(("A0", "P", "A1")):
                    wg = _WG[(cls, blk)]
                    nc.tensor.matmul(red[:], wall[:, wg:wg + 4],
                                     pk[:, bi * 512:(bi + 1) * 512],
                                     start=(k == 0 and bi == 0),
                                     stop=(k == 7 and bi == 2))
            eall = ep.tile([4, 512], dt.float32, tag="eall")
            nc.scalar.copy(eall[:], red[:])

            # ---- stage E: finish: out = E_L + Eout * P ----
            po = ps_misc.tile([128, 512], dt.float32, tag="misc")
            for s in range(4):
                nc.tensor.transpose(po[:, s * 4:s * 4 + 4],
                                    eall[:, s * 128:(s + 1) * 128], id4[:])
            pexp = op.tile([128, 16], dt.float32, tag="pexp")
            nc.scalar.activation(pexp[:], tit[:], _mb.ActivationFunctionType.Exp,
                                 bias=bias_t[:], scale=LN10_10)
            ot = op.tile([128, 16], dt.float32, tag="ot")
            for cc in range(4):
                nc.vector.tensor_mul(ot[:, cc:16:4], po[:, cc:16:4], pexp[:, 0:16:4])
            # E_L: out col 4s+2m+comp += nat_comp[:, 82s+2L+m]
            for s in range(4):
                nc.vector.tensor_add(ot[:, 4 * s:4 * s + 4:2], ot[:, 4 * s:4 * s + 4:2],
                                     nat["r"][:, 82 * s + 2 * L:82 * s + 2 * L + 2])
                nc.vector.tensor_add(ot[:, 4 * s + 1:4 * s + 4:2], ot[:, 4 * s + 1:4 * s + 4:2],
                                     nat["i"][:, 82 * s + 2 * L:82 * s + 2 * L + 2])
            nc.sync.dma_start(out=out_d[r0:r0 + 128, :], in_=ot[:])

    nc.compile()
    return nc


_CACHE = {}


def kernel(xr, xi, task_info, Wr, Wi):
    from concourse.bass_utils import run_bass_kernel_spmd

    xr = np.ascontiguousarray(np.asarray(xr, dtype=np.float32)).reshape(B // 4, 328)
    xi = np.ascontiguousarray(np.asarray(xi, dtype=np.float32)).reshape(B // 4, 328)
    task_info = np.ascontiguousarray(np.asarray(task_info, dtype=np.float32)).reshape(B // 4, 16)
    gm = _build_consts()
    Wr32 = np.asarray(Wr, dtype=np.float32)
    Wi32 = np.asarray(Wi, dtype=np.float32)
    wall = _build_wall(Wr32, Wi32)
    id128 = np.eye(128, dtype=np.float32)
    id4 = np.eye(4, dtype=np.float32)

    if "nc" not in _CACHE:
        _CACHE["nc"] = _build_kernel()
    nc = _CACHE["nc"]

    rows = BC // 4
    in_maps = []
    for core in range(NCORES):
        s = slice(core * rows, (core + 1) * rows)
        in_maps.append({
            "xr": xr[s], "xi": xi[s], "ti": task_info[s],
            "gn": gm["n"], "gm": gm["m"], "gmn": gm["mn"],
            "ffp": _build_ffold(1.0), "ffm": _build_ffold(-1.0),
            "wall": wall, "id128": id128, "id4": id4,
        })
    res = run_bass_kernel_spmd(nc, in_maps, list(range(NCORES)))
    outs = [res.results[i]["out"] for i in range(NCORES)]
    full = np.concatenate(outs, axis=0)  # [B//4, 16]
    return full.reshape(B, NMODES, 2).astype(np.float32)
